# revision 63
# baseline (speedup 1.0000x reference)
"""Trainium2 Bass kernel for a 6-layer post-LN transformer encoder.

Sharding: data-parallel over batch — B=8, one batch element per NeuronCore,
no collectives.  Each core runs the full 6-layer encoder on its [S, D] slice.

Device-side layout: activations are kept feature-major ([D, S], "xT") in SBUF
so that every matmul can use input-major weights and PE contracts over the
partition dim:  out[m, n] = sum_k lhsT[k, m] * rhs[k, n]

v3/v4 structure (vs the v2 baseline; HW 1.78ms vs 2.08ms measured):
  - FFN1/FFN2 run nh-half-outer (W1 streamed once per half) so the LN1/LN2
    stat chains software-pipeline under PE work of the other half, and the
    prev layer's LN2 finalize overlaps the next layer's V projections.
  - LayerNorm split into stats() (PE row-sums + 6-op ACT/DVE chain, eps
    folded into the Ln bias) and finalize() (k=2 D-matmul + 2 DVE ops per
    d-tile).  m*r rows live in two manually alternated const tiles (no WAR
    serialization).  The m_sb PSUM copy chain is one op shorter than v2.
  - Attention: ctx PSUM keeps the softmax denominator at row 64, but the
    reciprocal lands at partition 0 of a row tile so the 1/denom broadcast
    runs on the idle GPSIMD engine (no k=1 broadcast matmuls, no bc_sb
    copies).  ctx head-pairs are packed into [128, S] tiles so Wo contracts
    k=128 (half the Wo matmuls).  QK PSUM eviction rides ACT.
  - Softmax exp split ACT/DVE: t-steps in SCH_T use a Schraudolph bf16-bit
    exp on DVE, the rest use the ACT table Exp.  exp tiles are 6-deep, and
    scores/exp for step t+1 are emitted ahead of step t's ctx matmuls so
    the PE never idles on exp latency.
  - Weights moving through the PE stay fp32r (neuronxcc forbids mixed
    fp32r x bf16 matmuls; fp32r streams at full rate for free>=256), Wo/W2
    are bf16 stationaries.

HW notes (measured, this container): per-matmul instruction overhead in
kernel context is ~0 (splitting FFN2 into twice as many free-256 matmuls
was free), but each extra ACT op / PSUM tile-group costs ~1us — a per-head
score-tile split (+64 ACT ops/layer) regressed 43%.  Total matmul output
rows x 0.83ns (1.2 GHz PE) tracks measured wall time closely, so reducing
rows or raising the sustained PE clock are the levers that matter.
"""

import numpy as np

L, H, D, DK, DFF = 6, 8, 512, 64, 2048
B, S = 8, 1024
EPS = 1e-5
P = 128
NDT = D // P        # 4  d-tiles
NST = S // P        # 8  s/t-tiles
NFT = DFF // P      # 16 dff-tiles
NPAIR = H // 2      # 4  head pairs
NH = S // 512       # 2  n-halves (512-wide fp32 matmul free dim)
SCALE = 1.0 / np.sqrt(np.float32(DK))
FCH = 2             # W1 streamed in chunks of 2 dff-tiles
SCH_A = float(128.0 / np.log(2.0)) * float(SCALE)
SCH_B = float(127 * 128 - 486411.0 / 65536.0)
SCH_T = (2, 4, 6)  # t-steps whose softmax exp runs on DVE (Schraudolph)
FF2_SPLIT = 2  # FFN2 mms split into this many free-chunks (measured faster)

_CACHE = {}


def _round_fp32r(a: np.ndarray) -> np.ndarray:
    """Round fp32 to the fp32r grid (11 mantissa bits), round-to-nearest-even."""
    u = np.ascontiguousarray(a, dtype=np.float32).view(np.uint32)
    r = (u + np.uint32(0x7FF) + ((u >> np.uint32(12)) & np.uint32(1))) & np.uint32(
        0xFFFFF000
    )
    return r.view(np.float32)


def _build_nc():
    import concourse.bass as bass
    import concourse.bacc as bacc
    import concourse.tile as tile
    from concourse import mybir

    fp32 = mybir.dt.float32
    fp32r = mybir.dt.float32r
    bf16 = mybir.dt.bfloat16
    i16 = mybir.dt.int16
    AF = mybir.ActivationFunctionType
    OP = mybir.AluOpType

    class _Bacc(bacc.Bacc):
        # Exp (softmax) and Ln (layernorm rstd) live in different default
        # activation-table sets, causing ~50 table-load thrashes (~2.7us
        # each). Restrict both to natural_log_exp_and_others (which holds
        # both, plus Copy/Square/Relu) so one load serves the whole kernel.
        def insert_act_table_loads(self):
            from concourse.hw_specs import get_activation_tables
            import bass_rust as _bass_rust

            has_act = any(
                isinstance(i, mybir.InstActivation)
                for b in self.main_func.blocks
                for i in b.instructions
            )
            if not has_act:
                return
            AF2 = mybir.ActivationFunctionType
            tables = []
            for name, fns in get_activation_tables(self.m.arch).items():
                if name != "natural_log_exp_and_others":
                    fns = fns - {AF2.Exp, AF2.Ln}
                tables.append((name, fns))
            _bass_rust.insert_act_table_loads(self, tables)

    nc = _Bacc()

    def mm(out, lhsT, rhs, **kw):
        # fp32-typed APs go through the PE as fp32r (1 row/cycle at free>=256)
        def c(ap):
            return ap.bitcast(fp32r) if ap.dtype == fp32 else ap

        return nc.tensor.matmul(out, c(lhsT), c(rhs), **kw)

    def f(ap):
        # view a float32r tile as plain fp32 for DVE/ACT reads
        return ap.bitcast(fp32) if ap.dtype == fp32r else ap

    x_d = nc.declare_dram_parameter("x", [NDT, P, S], fp32r, isOutput=False)
    wq_d = nc.declare_dram_parameter(
        "wq", [L, P, NDT, NPAIR, P], fp32r, isOutput=False
    )
    wk_d = nc.declare_dram_parameter(
        "wk", [L, P, NDT, NPAIR, P], fp32r, isOutput=False
    )
    wv_d = nc.declare_dram_parameter("wv", [L, P, NDT, H * DK], fp32r, isOutput=False)
    wo_d = nc.declare_dram_parameter("wo", [L, P, NPAIR, NDT, P], bf16, isOutput=False)
    w1_d = nc.declare_dram_parameter(
        "w1", [L, NFT // FCH, P, NDT, FCH, P], fp32r, isOutput=False
    )
    w2_d = nc.declare_dram_parameter("w2", [L, P, NDT, NFT, P], bf16, isOutput=False)
    g1_d = nc.declare_dram_parameter("g1", [L, P, NDT], fp32, isOutput=False)
    g2_d = nc.declare_dram_parameter("g2", [L, P, NDT], fp32, isOutput=False)
    gb1_d = nc.declare_dram_parameter("gb1", [L, 2, NDT, P], fp32r, isOutput=False)
    gb2_d = nc.declare_dram_parameter("gb2", [L, 2, NDT, P], fp32r, isOutput=False)
    b1_d = nc.declare_dram_parameter("b1", [L, P, NFT], fp32, isOutput=False)
    b2_d = nc.declare_dram_parameter("b2", [L, P, NDT], fp32, isOutput=False)
    ones16_d = nc.declare_dram_parameter("ones16", [P, 16], bf16, isOutput=False)
    mro_d = nc.declare_dram_parameter("mro", [2, 512], fp32r, isOutput=False)
    out_d = nc.declare_dram_parameter("out", [NDT, P, S], fp32, isOutput=True)

    with tile.TileContext(nc) as tc:
        from contextlib import ExitStack

        with ExitStack() as ctx:
            ec = ctx.enter_context
            ec(
                nc.allow_low_precision(
                    reason="bf16/fp32r matmul operands; fp32 PSUM accumulation"
                )
            )
            # --- SBUF pools ---
            const_p = ec(tc.tile_pool(name="const", bufs=1))
            wts_p = ec(tc.tile_pool(name="wts", bufs=2))
            wbig_p = ec(tc.tile_pool(name="wbig", bufs=1))
            w1c_p = ec(tc.tile_pool(name="w1c", bufs=2))
            xt_p = ec(tc.tile_pool(name="xt", bufs=5))
            qk_p = ec(tc.tile_pool(name="qk", bufs=2))
            v_p = ec(tc.tile_pool(name="v", bufs=8))
            exp_p = ec(tc.tile_pool(name="exp", bufs=7))
            ctx_p = ec(tc.tile_pool(name="ctxp", bufs=4))
            mha_p = ec(tc.tile_pool(name="mha", bufs=4))
            ff1_p = ec(tc.tile_pool(name="ff1", bufs=16))
            sq_p = ec(tc.tile_pool(name="sq", bufs=3))
            bcs_p = ec(tc.tile_pool(name="bcs", bufs=2))
            rows_p = ec(tc.tile_pool(name="rows", bufs=2))
            # --- PSUM pools: 4 + 2 + 2 = 8 banks ---
            # sc:  [128,1024] score-pair tiles (2 banks each) / FFN1 halves
            # acc: attention ctx accumulators / FFN2 groups / LN stat rows
            # mm:  short-lived k<=4 matmul outputs (QKV/V/Wo/LN-D)
            pp_sc = ec(tc.tile_pool(name="pp_sc", bufs=2, space="PSUM"))
            pp_acc = ec(tc.tile_pool(name="pp_acc", bufs=2, space="PSUM"))
            pp_mm = ec(tc.tile_pool(name="pp_mm", bufs=2, space="PSUM"))

            # constants
            ones16 = const_p.tile([P, 16], bf16)
            nc.sync.dma_start(out=ones16, in_=ones16_d[:, :])
            zero_col = const_p.tile([P, 1], fp32)
            nc.vector.memset(zero_col, 0.0)
            eps_col = const_p.tile([P, 1], fp32)
            nc.vector.memset(eps_col, float(EPS))
            invd_col = const_p.tile([P, 1], fp32)
            nc.vector.memset(invd_col, 1.0 / D)
            # rhs of the LN D-matmul: row0 = m*r (rewritten per LN half),
            # row1 = 1.  Two tiles alternated manually to avoid WAR chains.
            mr_a = const_p.tile([2, 512], fp32r)
            nc.sync.dma_start(out=mr_a, in_=mro_d[:, :])
            mr_b = const_p.tile([2, 512], fp32r)
            nc.sync.dma_start(out=mr_b, in_=mro_d[:, :])
            mr_ab = (mr_a, mr_b)

            # layer-0 input
            xt = []
            for dt in range(NDT):
                t = xt_p.tile([P, S], fp32r, tag="xt")
                nc.sync.dma_start(out=t, in_=x_d[dt])
                xt.append(t)

            # ---------------- layernorm helpers ------------------------------
            def ln_stats(ytiles, nh, mr_idx, pre_sq=None):
                """Row stats for half nh: PE p1/p2 sums + serial chain.

                Returns (mrt, bcr) for ln_finalize.  PE cost: 8 matmuls; the
                6-op ACT/DVE chain runs while PE continues other work.
                """
                ssl = slice(nh * 512, (nh + 1) * 512)
                p1 = pp_acc.tile([1, 512], fp32, tag="acc", name="p1")
                for dt in range(NDT):
                    mm(
                        p1,
                        invd_col,
                        ytiles[dt][:, ssl],
                        start=(dt == 0),
                        stop=(dt == NDT - 1),
                    )
                p2 = pp_acc.tile([1, 512], fp32, tag="acc", name="p2")
                for dt in range(NDT):
                    if pre_sq is not None and pre_sq[dt] is not None:
                        sqt = pre_sq[dt]
                    else:
                        sqt = sq_p.tile([P, 512], fp32r, tag="sq", name="sqt")
                        nc.scalar.activation(
                            sqt, f(ytiles[dt][:, ssl]), AF.Square, bias=zero_col
                        )
                    mm(
                        p2,
                        invd_col,
                        sqt,
                        start=(dt == 0),
                        stop=(dt == NDT - 1),
                    )
                m_sb = rows_p.tile([1, 512], fp32, tag="m_sb")
                nc.scalar.copy(m_sb, p1)
                msq = rows_p.tile([1, 512], fp32, tag="msq")
                nc.scalar.activation(msq, m_sb, AF.Square, bias=zero_col[0:1])
                var = rows_p.tile([1, 512], fp32, tag="var")
                nc.vector.tensor_sub(var, p2, msq)
                # rstd = exp(-0.5 * ln(var + eps))
                nc.scalar.activation(var, var, AF.Ln, bias=eps_col[0:1])
                r_sb = rows_p.tile([1, 512], fp32r, tag="r_sb")
                nc.scalar.activation(
                    r_sb, var, AF.Exp, bias=zero_col[0:1], scale=-0.5
                )
                mrt = mr_ab[mr_idx]
                nc.vector.tensor_mul(mrt[0:1], m_sb, f(r_sb))
                bcr = bcs_p.tile([P, 512], fp32, tag="bcr", name="bcr")
                nc.gpsimd.partition_broadcast(bcr, f(r_sb), channels=P)
                return mrt, bcr

            def ln_finalize(ytiles, nh, g_t, gb_t, stats):
                # out = (y*g)*bc(r) + D,   D = (-g) (x) (m*r) + b  (k=2 MM)
                mrt, bcr = stats
                ssl = slice(nh * 512, (nh + 1) * 512)
                for dt in range(NDT):
                    Dt = pp_mm.tile([P, 512], fp32, tag="mm", name="Dt")
                    mm(Dt, gb_t[:, dt, :], mrt)
                    nc.vector.scalar_tensor_tensor(
                        ytiles[dt][:, ssl],
                        f(ytiles[dt][:, ssl]),
                        g_t[:, dt : dt + 1],
                        bcr,
                        OP.mult,
                        OP.mult,
                    )
                    nc.vector.tensor_add(
                        ytiles[dt][:, ssl], f(ytiles[dt][:, ssl]), Dt
                    )

            # state carried across the layer loop for LN2 pipelining
            carry = {}

            for l in range(L):
                # ---------------- weight loads (wv/wq/wk first) -------------
                wv_t = wbig_p.tile([P, NDT, H * DK], fp32r, tag="wv")
                nc.sync.dma_start(out=wv_t, in_=wv_d[l])
                wq_t = wbig_p.tile([P, NDT, NPAIR, P], fp32r, tag="wq")
                nc.sync.dma_start(out=wq_t, in_=wq_d[l])
                wk_t = wbig_p.tile([P, NDT, NPAIR, P], fp32r, tag="wk")
                nc.sync.dma_start(out=wk_t, in_=wk_d[l])
                g1_t = wts_p.tile([P, NDT], fp32, tag="g1")
                nc.sync.dma_start(out=g1_t, in_=g1_d[l])
                g2_t = wts_p.tile([P, NDT], fp32, tag="g2")
                nc.sync.dma_start(out=g2_t, in_=g2_d[l])
                gb1_t = wts_p.tile([2, NDT, P], fp32r, tag="gb1")
                nc.sync.dma_start(out=gb1_t, in_=gb1_d[l])
                gb2_t = wts_p.tile([2, NDT, P], fp32r, tag="gb2")
                nc.sync.dma_start(out=gb2_t, in_=gb2_d[l])
                b1_t = wts_p.tile([P, NFT], fp32, tag="b1")
                nc.sync.dma_start(out=b1_t, in_=b1_d[l])
                b2_t = wts_p.tile([P, NDT], fp32, tag="b2")
                nc.sync.dma_start(out=b2_t, in_=b2_d[l])
                wo_t = wbig_p.tile([P, NPAIR, NDT, P], bf16, tag="wo")
                nc.sync.dma_start(out=wo_t, in_=wo_d[l])
                w2_t = wbig_p.tile([P, NDT, NFT, P], bf16, tag="w2")
                nc.sync.dma_start(out=w2_t, in_=w2_d[l])

                # ---------------- prev-layer LN2 finalize, pipelined with V -
                if carry:
                    ln_finalize(xt, 0, carry["g2"], carry["gb2"], carry["s0"])

                # ---------------- V = x @ Wv  (col 64 = ones -> psum row 64 =
                # softmax denominator; rows 0..63 per head = ctx) ------------
                v_tiles = []
                for st in range(NST):
                    if st == 4 and carry:
                        ln_finalize(xt, 1, carry["g2"], carry["gb2"], carry["s1"])
                        carry.clear()
                    vt = v_p.tile([P, H, DK + 1], bf16, tag="v")
                    nc.sync.dma_start(out=vt[:, :, DK], in_=ones16_d[:, 0:H])
                    ps = pp_mm.tile([P, 512], fp32, tag="mm")
                    for dt in range(NDT):
                        mm(
                            ps,
                            xt[dt][:, st * P : (st + 1) * P],
                            wv_t[:, dt, :],
                            start=(dt == 0),
                            stop=(dt == NDT - 1),
                        )
                    nc.vector.tensor_copy(
                        vt[:, :, 0:DK], ps.rearrange("p (h k) -> p h k", h=H)
                    )
                    v_tiles.append(vt)
                if carry:  # layer 0 never reaches here with carry set
                    ln_finalize(xt, 1, carry["g2"], carry["gb2"], carry["s1"])
                    carry.clear()

                # ---------------- QKV (per head-pair, JIT) + attention ------
                qt = [None] * NPAIR
                kt = [None] * NPAIR
                ctx_pairs = []
                for pr in range(NPAIR):
                    cp = ctx_p.tile([P, S], bf16, tag="ctx", name="cp")
                    ctx_pairs.append(cp)

                def make_qk(pr, w_t, tag):
                    # PSUM->SBUF eviction rides ACT (GPSIMD can't read PSUM)
                    # so the attention-loop DVE burst doesn't delay the next
                    # pair's score matmuls; one more exp step moves to DVE in
                    # exchange (SCH_T has 3 entries).
                    dst = qk_p.tile([P, S], bf16, tag=tag)
                    for nh in range(NH):
                        ps = pp_mm.tile([P, 512], fp32, tag="mm")
                        for dt in range(NDT):
                            mm(
                                ps,
                                w_t[:, dt, pr, :],
                                xt[dt][:, nh * 512 : (nh + 1) * 512],
                                start=(dt == 0),
                                stop=(dt == NDT - 1),
                            )
                        nc.scalar.copy(dst[:, nh * 512 : (nh + 1) * 512], ps)
                    return dst

                qt[0] = make_qk(0, wq_t, "qt")
                kt[0] = make_qk(0, wk_t, "kt")
                for pr in range(NPAIR):
                    if pr + 1 < NPAIR:
                        # emit next pair's QKV before this pair's attention so
                        # its DVE evicts sit ahead of the attention DVE burst
                        qt[pr + 1] = make_qk(pr + 1, wq_t, "qt")
                        kt[pr + 1] = make_qk(pr + 1, wk_t, "kt")

                    hA, hB = 2 * pr, 2 * pr + 1
                    for nh in range(NH):
                        ssl = slice(nh * 512, (nh + 1) * 512)
                        psA = pp_acc.tile([65, 512], fp32, tag="acc")
                        psB = pp_acc.tile([65, 512], fp32, tag="acc")

                        es = [None] * NST

                        def emit_sc_exp(t):
                            # scores + exp for step t (emitted one step ahead
                            # of the ctx matmuls so the PE never waits on the
                            # exp engines)
                            tsl = slice(t * P, (t + 1) * P)
                            sc = pp_sc.tile([P, 1024], fp32, tag="sc")
                            mm(sc[:, 0:512], kt[pr][0:64, tsl], qt[pr][0:64, ssl])
                            mm(
                                sc[:, 512:1024],
                                kt[pr][64:128, tsl],
                                qt[pr][64:128, ssl],
                            )
                            if t in SCH_T:
                                # Schraudolph exp on DVE: bf16 bit pattern =
                                # int16(A*score + B); numerator and denominator
                                # share e, so softmax stays normalized.
                                ei = exp_p.tile([P, 1024], i16, tag="exp")
                                nc.vector.tensor_scalar(
                                    out=ei,
                                    in0=sc,
                                    scalar1=SCH_A,
                                    scalar2=SCH_B,
                                    op0=OP.mult,
                                    op1=OP.add,
                                )
                                es[t] = ei.bitcast(bf16)
                            else:
                                e = exp_p.tile([P, 1024], bf16, tag="exp")
                                nc.scalar.activation(
                                    e, sc, AF.Exp, bias=zero_col, scale=float(SCALE)
                                )
                                es[t] = e

                        emit_sc_exp(0)
                        for t in range(NST):
                            if t + 1 < NST:
                                emit_sc_exp(t + 1)
                            e = es[t]
                            mm(
                                psA,
                                v_tiles[t][:, hA, :],
                                e[:, 0:512],
                                start=(t == 0),
                                stop=(t == NST - 1),
                            )
                            mm(
                                psB,
                                v_tiles[t][:, hB, :],
                                e[:, 512:1024],
                                start=(t == 0),
                                stop=(t == NST - 1),
                            )
                        # normalize: denom at row 64, ctx rows 0..63.  The
                        # reciprocal lands at partition 0 of rX so the GPSIMD
                        # broadcast (which reads partition 0) can fan it out;
                        # ctx goes into the pair tile rows [64*h : 64*h+64].
                        for hoff, psX in ((0, psA), (64, psB)):
                            rX = rows_p.tile([1, 512], fp32, tag="rX")
                            nc.vector.reciprocal(rX, psX[64:65])
                            bcd = bcs_p.tile([64, 512], fp32, tag="bcd", name="bcd")
                            nc.gpsimd.partition_broadcast(bcd, rX, channels=64)
                            nc.vector.tensor_mul(
                                ctx_pairs[pr][hoff : hoff + 64, ssl],
                                psX[0:64],
                                bcd,
                            )

                # ---------------- Wo + residual -> y (pre-LN1), LN1 stats
                # pipelined between the halves ------------------------------
                y = []
                for mt in range(NDT):
                    yt = mha_p.tile([P, S], fp32r, tag="mha")
                    y.append(yt)
                stats1 = [None, None]
                for nh in range(NH):
                    ssl = slice(nh * 512, (nh + 1) * 512)
                    for mt in range(NDT):
                        if nh == 1 and mt == 1:
                            stats1[0] = ln_stats(y, 0, 0)
                        ps = pp_mm.tile([P, 512], fp32, tag="mm")
                        for pr in range(NPAIR):
                            mm(
                                ps,
                                wo_t[:, pr, mt, :],
                                ctx_pairs[pr][:, ssl],
                                start=(pr == 0),
                                stop=(pr == NPAIR - 1),
                            )
                        nc.vector.tensor_add(y[mt][:, ssl], ps, f(xt[mt][:, ssl]))
                ln_finalize(y, 0, g1_t, gb1_t, stats1[0])
                stats1[1] = ln_stats(y, 1, 1)

                # ---------------- FFN1 (nh-outer; W1 streamed per pass) -----
                f1s = [None] * NFT
                for nh in range(NH):
                    ssl = slice(nh * 512, (nh + 1) * 512)
                    for fc in range(NFT // FCH):
                        w1_t = w1c_p.tile([P, NDT, FCH, P], fp32r, tag="w1c")
                        nc.sync.dma_start(out=w1_t, in_=w1_d[l, fc])
                        for fi in range(FCH):
                            ft = fc * FCH + fi
                            if nh == 0 and ft == 3:
                                ln_finalize(y, 1, g1_t, gb1_t, stats1[1])
                            if nh == 0:
                                f1s[ft] = ff1_p.tile(
                                    [P, S], bf16, tag="ff1", name="f1"
                                )
                            ps = pp_sc.tile([P, 512], fp32, tag="sc", name="psf")
                            for dt in range(NDT):
                                mm(
                                    ps,
                                    w1_t[:, dt, fi, :],
                                    y[dt][:, ssl],
                                    start=(dt == 0),
                                    stop=(dt == NDT - 1),
                                )
                            nc.scalar.activation(
                                f1s[ft][:, ssl],
                                ps,
                                AF.Relu,
                                bias=b1_t[:, ft : ft + 1],
                            )

                # ---------------- FFN2 + residual -> z, LN2 stats pipelined -
                z = []
                for mt in range(NDT):
                    zt = xt_p.tile([P, S], fp32r, tag="xt")
                    z.append(zt)
                stats2 = [None, None]
                sq_nh1 = [None] * NDT
                for nh in range(NH):
                    ssl = slice(nh * 512, (nh + 1) * 512)
                    for mt in range(NDT):
                        if nh == 1 and mt == 1:
                            stats2[0] = ln_stats(z, 0, 0)
                        ff2 = pp_acc.tile([P, 512], fp32, tag="acc")
                        w = 512 // FF2_SPLIT
                        for hb in range(FF2_SPLIT):
                            for ft in range(NFT):
                                mm(
                                    ff2[:, hb * w : (hb + 1) * w],
                                    w2_t[:, mt, ft, :],
                                    f1s[ft][
                                        :, nh * 512 + hb * w : nh * 512 + (hb + 1) * w
                                    ],
                                    start=(ft == 0),
                                    stop=(ft == NFT - 1),
                                )
                        nc.vector.scalar_tensor_tensor(
                            z[mt][:, ssl],
                            ff2,
                            b2_t[:, mt : mt + 1],
                            f(y[mt][:, ssl]),
                            OP.add,
                            OP.add,
                        )
                        if nh == 1 and mt < NDT - 1:
                            # pre-square z (ACT queue only) so the LN2-nh1
                            # chain starts as soon as FFN2 retires
                            sqe = sq_p.tile([P, 512], fp32r, tag="sq", name="sqe")
                            nc.scalar.activation(
                                sqe, f(z[mt][:, ssl]), AF.Square, bias=zero_col
                            )
                            sq_nh1[mt] = sqe
                stats2[1] = ln_stats(z, 1, 1, pre_sq=sq_nh1)
                carry = {
                    "s0": stats2[0],
                    "s1": stats2[1],
                    "g2": g2_t,
                    "gb2": gb2_t,
                }
                xt = z

            # tail: finalize the last layer's LN2, streaming halves out
            ln_finalize(xt, 0, carry["g2"], carry["gb2"], carry["s0"])
            for dt in range(NDT):
                nc.sync.dma_start(
                    out=out_d[dt][:, 0:512], in_=f(xt[dt][:, 0:512])
                )
            ln_finalize(xt, 1, carry["g2"], carry["gb2"], carry["s1"])
            for dt in range(NDT):
                nc.sync.dma_start(
                    out=out_d[dt][:, 512:1024], in_=f(xt[dt][:, 512:1024])
                )

    return nc


def _prep_weights(Wq, Wk, Wv, Wo, ln1_g, ln1_b, W1, b1, W2, b2, ln2_g, ln2_b):
    import ml_dtypes

    f = np.float32
    bf = ml_dtypes.bfloat16

    def qk_r(W):  # [L,H,D,DK] -> [L, 128, NDT, NPAIR, 128] fp32r
        return _round_fp32r(
            W.reshape(L, NPAIR, 2, NDT, P, DK)
            .transpose(0, 4, 3, 1, 2, 5)
            .reshape(L, P, NDT, NPAIR, P)
            .astype(f)
        )

    wv_r = _round_fp32r(
        Wv.transpose(0, 2, 1, 3)  # [L, D, H, DK]
        .reshape(L, NDT, P, H * DK)
        .transpose(0, 2, 1, 3)
        .reshape(L, P, NDT, H * DK)
        .astype(f)
    )
    # Wo packed for k=128 head-pair steps: [l, c, pr, mt, fcol] where
    # c = (h_in_pair * 64 + dk) and the input feature is (2*pr+h_in_pair)*64+dk
    wo_r = (
        Wo.reshape(L, NPAIR, 2 * DK, NDT, P)
        .transpose(0, 2, 1, 3, 4)
        .astype(bf)
    )
    w1_r = _round_fp32r(
        W1.reshape(L, NDT, P, NFT // FCH, FCH, P)
        .transpose(0, 3, 2, 1, 4, 5)
        .astype(f)
    )
    # w2[l, p, mt, ft, fcol] = W2[l, ft*128+p, mt*128+fcol]
    w2_r = W2.reshape(L, NFT, P, NDT, P).transpose(0, 2, 3, 1, 4).astype(bf)

    def ln_r(v, n):  # [L, n*128] -> [L, 128, n]
        return np.ascontiguousarray(v.reshape(L, n, P).transpose(0, 2, 1).astype(f))

    def gb_r(g, b):  # [L, D] x2 -> [L, 2, NDT, P] rows (-g, b), fp32r
        gneg = (-np.asarray(g, dtype=f)).reshape(L, 1, NDT, P)
        bb = np.asarray(b, dtype=f).reshape(L, 1, NDT, P)
        return _round_fp32r(np.concatenate([gneg, bb], axis=1))

    return {
        "wq": qk_r(Wq),
        "wk": qk_r(Wk),
        "wv": wv_r,
        "wo": wo_r,
        "w1": w1_r,
        "w2": w2_r,
        "g1": ln_r(ln1_g, NDT),
        "g2": ln_r(ln2_g, NDT),
        "gb1": gb_r(ln1_g, ln1_b),
        "gb2": gb_r(ln2_g, ln2_b),
        "b1": ln_r(b1, NFT),
        "b2": ln_r(b2, NDT),
    }


def get_nc():
    if "nc" not in _CACHE:
        nc = _build_nc()
        if not nc.is_finalized():
            nc.finalize()
        _CACHE["nc"] = nc
    return _CACHE["nc"]


def make_in_maps(**inputs):
    import ml_dtypes

    inputs = {k: np.asarray(v, dtype=np.float32) for k, v in inputs.items()}
    x = inputs.pop("x")
    wmap = _prep_weights(**inputs)
    in_maps = []
    wmap["ones16"] = np.ones((P, 16), dtype=ml_dtypes.bfloat16)
    mro = np.zeros((2, 512), dtype=np.float32)
    mro[1, :] = 1.0
    wmap["mro"] = mro
    for b in range(B):
        xt = _round_fp32r(x[b].T.reshape(NDT, P, S))
        in_maps.append({"x": xt, **wmap})
    return in_maps


def kernel(**inputs) -> np.ndarray:
    from concourse.bass_utils import run_bass_kernel_spmd

    nc = get_nc()
    in_maps = make_in_maps(**inputs)
    res = run_bass_kernel_spmd(nc, in_maps, core_ids=list(range(B)))
    out = np.empty((B, S, D), dtype=np.float32)
    for b in range(B):
        out[b] = res.results[b]["out"].reshape(D, S).T
    return out


if __name__ == "__main__":
    rng = np.random.default_rng(0)
    ins = {
        "x": rng.standard_normal((B, S, D), dtype=np.float32),
        "Wq": rng.standard_normal((L, H, D, DK), dtype=np.float32) * 0.02,
        "Wk": rng.standard_normal((L, H, D, DK), dtype=np.float32) * 0.02,
        "Wv": rng.standard_normal((L, H, D, DK), dtype=np.float32) * 0.02,
        "Wo": rng.standard_normal((L, D, D), dtype=np.float32) * 0.02,
        "ln1_g": np.ones((L, D), np.float32),
        "ln1_b": np.zeros((L, D), np.float32),
        "W1": rng.standard_normal((L, D, DFF), dtype=np.float32) * 0.02,
        "b1": np.zeros((L, DFF), np.float32),
        "W2": rng.standard_normal((L, DFF, D), dtype=np.float32) * 0.02,
        "b2": np.zeros((L, D), np.float32),
        "ln2_g": np.ones((L, D), np.float32),
        "ln2_b": np.zeros((L, D), np.float32),
    }
    out = kernel(**ins)
    print(out.shape, out.dtype, np.abs(out).mean())


# revision 64
# speedup vs baseline: 1.0535x; 1.0535x over previous
"""Trainium2 Bass kernel for a 6-layer post-LN transformer encoder.

Sharding: data-parallel over batch — B=8, one batch element per NeuronCore,
no collectives.  Each core runs the full 6-layer encoder on its [S, D] slice.

Device-side layout: activations are kept feature-major ([D, S], "xT") in SBUF
so that every matmul can use input-major weights and PE contracts over the
partition dim:  out[m, n] = sum_k lhsT[k, m] * rhs[k, n]

v3/v4 structure (vs the v2 baseline; HW 1.78ms vs 2.08ms measured):
  - FFN1/FFN2 run nh-half-outer (W1 streamed once per half) so the LN1/LN2
    stat chains software-pipeline under PE work of the other half, and the
    prev layer's LN2 finalize overlaps the next layer's V projections.
  - LayerNorm split into stats() (PE row-sums + 6-op ACT/DVE chain, eps
    folded into the Ln bias) and finalize() (k=2 D-matmul + 2 DVE ops per
    d-tile).  m*r rows live in two manually alternated const tiles (no WAR
    serialization).  The m_sb PSUM copy chain is one op shorter than v2.
  - Attention: ctx PSUM keeps the softmax denominator at row 64, but the
    reciprocal lands at partition 0 of a row tile so the 1/denom broadcast
    runs on the idle GPSIMD engine (no k=1 broadcast matmuls, no bc_sb
    copies).  ctx head-pairs are packed into [128, S] tiles so Wo contracts
    k=128 (half the Wo matmuls).  QK PSUM eviction rides ACT.
  - Softmax exp split ACT/DVE: t-steps in SCH_T use a Schraudolph bf16-bit
    exp on DVE, the rest use the ACT table Exp.  exp tiles are 6-deep, and
    scores/exp for step t+1 are emitted ahead of step t's ctx matmuls so
    the PE never idles on exp latency.
  - Weights moving through the PE stay fp32r (neuronxcc forbids mixed
    fp32r x bf16 matmuls; fp32r streams at full rate for free>=256), Wo/W2
    are bf16 stationaries.

HW notes (measured, this container): per-matmul instruction overhead in
kernel context is ~0 (splitting FFN2 into twice as many free-256 matmuls
was free), but each extra ACT op / PSUM tile-group costs ~1us — a per-head
score-tile split (+64 ACT ops/layer) regressed 43%.  Total matmul output
rows x 0.83ns (1.2 GHz PE) tracks measured wall time closely, so reducing
rows or raising the sustained PE clock are the levers that matter.
"""

import numpy as np

L, H, D, DK, DFF = 6, 8, 512, 64, 2048
B, S = 8, 1024
EPS = 1e-5
P = 128
NDT = D // P        # 4  d-tiles
NST = S // P        # 8  s/t-tiles
NFT = DFF // P      # 16 dff-tiles
NPAIR = H // 2      # 4  head pairs
NH = S // 512       # 2  n-halves (512-wide fp32 matmul free dim)
SCALE = 1.0 / np.sqrt(np.float32(DK))
FCH = 2             # W1 streamed in chunks of 2 dff-tiles
SCH_A = float(128.0 / np.log(2.0)) * float(SCALE)
SCH_B = float(127 * 128 - 486411.0 / 65536.0)
SCH_T = (3, 5, 7)  # t-steps whose softmax exp runs on DVE (Schraudolph)
FF2_SPLIT = 2  # FFN2 mms split into this many free-chunks (measured faster)

_CACHE = {}


def _round_fp32r(a: np.ndarray) -> np.ndarray:
    """Round fp32 to the fp32r grid (11 mantissa bits), round-to-nearest-even."""
    u = np.ascontiguousarray(a, dtype=np.float32).view(np.uint32)
    r = (u + np.uint32(0x7FF) + ((u >> np.uint32(12)) & np.uint32(1))) & np.uint32(
        0xFFFFF000
    )
    return r.view(np.float32)


def _build_nc():
    import concourse.bass as bass
    import concourse.bacc as bacc
    import concourse.tile as tile
    from concourse import mybir

    fp32 = mybir.dt.float32
    fp32r = mybir.dt.float32r
    bf16 = mybir.dt.bfloat16
    i16 = mybir.dt.int16
    AF = mybir.ActivationFunctionType
    OP = mybir.AluOpType

    class _Bacc(bacc.Bacc):
        # Exp (softmax) and Ln (layernorm rstd) live in different default
        # activation-table sets, causing ~50 table-load thrashes (~2.7us
        # each). Restrict both to natural_log_exp_and_others (which holds
        # both, plus Copy/Square/Relu) so one load serves the whole kernel.
        def insert_act_table_loads(self):
            from concourse.hw_specs import get_activation_tables
            import bass_rust as _bass_rust

            has_act = any(
                isinstance(i, mybir.InstActivation)
                for b in self.main_func.blocks
                for i in b.instructions
            )
            if not has_act:
                return
            AF2 = mybir.ActivationFunctionType
            tables = []
            for name, fns in get_activation_tables(self.m.arch).items():
                if name != "natural_log_exp_and_others":
                    fns = fns - {AF2.Exp, AF2.Ln}
                tables.append((name, fns))
            _bass_rust.insert_act_table_loads(self, tables)

    nc = _Bacc()

    def mm(out, lhsT, rhs, **kw):
        # fp32-typed APs go through the PE as fp32r (1 row/cycle at free>=256)
        def c(ap):
            return ap.bitcast(fp32r) if ap.dtype == fp32 else ap

        return nc.tensor.matmul(out, c(lhsT), c(rhs), **kw)

    def f(ap):
        # view a float32r tile as plain fp32 for DVE/ACT reads
        return ap.bitcast(fp32) if ap.dtype == fp32r else ap

    x_d = nc.declare_dram_parameter("x", [NDT, P, S], fp32r, isOutput=False)
    wq_d = nc.declare_dram_parameter(
        "wq", [L, P, NDT, NPAIR, P], fp32r, isOutput=False
    )
    wk_d = nc.declare_dram_parameter(
        "wk", [L, P, NDT, NPAIR, P], fp32r, isOutput=False
    )
    wv_d = nc.declare_dram_parameter("wv", [L, P, NDT, H * DK], fp32r, isOutput=False)
    wo_d = nc.declare_dram_parameter("wo", [L, P, NPAIR, NDT, P], bf16, isOutput=False)
    w1_d = nc.declare_dram_parameter(
        "w1", [L, NFT // FCH, P, NDT, FCH, P], fp32r, isOutput=False
    )
    w2_d = nc.declare_dram_parameter("w2", [L, P, NDT, NFT, P], bf16, isOutput=False)
    g1_d = nc.declare_dram_parameter("g1", [L, P, NDT], fp32, isOutput=False)
    g2_d = nc.declare_dram_parameter("g2", [L, P, NDT], fp32, isOutput=False)
    gb1_d = nc.declare_dram_parameter("gb1", [L, 2, NDT, P], fp32r, isOutput=False)
    gb2_d = nc.declare_dram_parameter("gb2", [L, 2, NDT, P], fp32r, isOutput=False)
    b1_d = nc.declare_dram_parameter("b1", [L, P, NFT], fp32, isOutput=False)
    b2_d = nc.declare_dram_parameter("b2", [L, P, NDT], fp32, isOutput=False)
    ones16_d = nc.declare_dram_parameter("ones16", [P, 16], bf16, isOutput=False)
    mro_d = nc.declare_dram_parameter("mro", [2, 512], fp32r, isOutput=False)
    out_d = nc.declare_dram_parameter("out", [NDT, P, S], fp32, isOutput=True)

    with tile.TileContext(nc) as tc:
        from contextlib import ExitStack

        with ExitStack() as ctx:
            ec = ctx.enter_context
            ec(
                nc.allow_low_precision(
                    reason="bf16/fp32r matmul operands; fp32 PSUM accumulation"
                )
            )
            # --- SBUF pools ---
            const_p = ec(tc.tile_pool(name="const", bufs=1))
            wts_p = ec(tc.tile_pool(name="wts", bufs=2))
            wbig_p = ec(tc.tile_pool(name="wbig", bufs=1))
            w1c_p = ec(tc.tile_pool(name="w1c", bufs=2))
            xt_p = ec(tc.tile_pool(name="xt", bufs=5))
            qk_p = ec(tc.tile_pool(name="qk", bufs=2))
            v_p = ec(tc.tile_pool(name="v", bufs=8))
            exp_p = ec(tc.tile_pool(name="exp", bufs=7))
            ctx_p = ec(tc.tile_pool(name="ctxp", bufs=4))
            mha_p = ec(tc.tile_pool(name="mha", bufs=4))
            ff1_p = ec(tc.tile_pool(name="ff1", bufs=16))
            sq_p = ec(tc.tile_pool(name="sq", bufs=3))
            bcs_p = ec(tc.tile_pool(name="bcs", bufs=2))
            rows_p = ec(tc.tile_pool(name="rows", bufs=2))
            # --- PSUM pools: 4 + 2 + 2 = 8 banks ---
            # sc:  [128,1024] score-pair tiles (2 banks each) / FFN1 halves
            # acc: attention ctx accumulators / FFN2 groups / LN stat rows
            # mm:  short-lived k<=4 matmul outputs (QKV/V/Wo/LN-D)
            pp_sc = ec(tc.tile_pool(name="pp_sc", bufs=2, space="PSUM"))
            pp_acc = ec(tc.tile_pool(name="pp_acc", bufs=2, space="PSUM"))
            pp_mm = ec(tc.tile_pool(name="pp_mm", bufs=2, space="PSUM"))

            # constants
            ones16 = const_p.tile([P, 16], bf16)
            nc.sync.dma_start(out=ones16, in_=ones16_d[:, :])
            zero_col = const_p.tile([P, 1], fp32)
            nc.vector.memset(zero_col, 0.0)
            eps_col = const_p.tile([P, 1], fp32)
            nc.vector.memset(eps_col, float(EPS))
            invd_col = const_p.tile([P, 1], fp32)
            nc.vector.memset(invd_col, 1.0 / D)
            # rhs of the LN D-matmul: row0 = m*r (rewritten per LN half),
            # row1 = 1.  Two tiles alternated manually to avoid WAR chains.
            mr_a = const_p.tile([2, 512], fp32r)
            nc.sync.dma_start(out=mr_a, in_=mro_d[:, :])
            mr_b = const_p.tile([2, 512], fp32r)
            nc.sync.dma_start(out=mr_b, in_=mro_d[:, :])
            mr_ab = (mr_a, mr_b)

            # layer-0 input
            xt = []
            for dt in range(NDT):
                t = xt_p.tile([P, S], fp32r, tag="xt")
                nc.sync.dma_start(out=t, in_=x_d[dt])
                xt.append(t)

            # ---------------- layernorm helpers ------------------------------
            def ln_stats(ytiles, nh, mr_idx, pre_sq=None):
                """Row stats for half nh: PE p1/p2 sums + serial chain.

                Returns (mrt, bcr) for ln_finalize.  PE cost: 8 matmuls; the
                6-op ACT/DVE chain runs while PE continues other work.
                """
                ssl = slice(nh * 512, (nh + 1) * 512)
                p1 = pp_acc.tile([1, 512], fp32, tag="acc", name="p1")
                for dt in range(NDT):
                    mm(
                        p1,
                        invd_col,
                        ytiles[dt][:, ssl],
                        start=(dt == 0),
                        stop=(dt == NDT - 1),
                    )
                p2 = pp_acc.tile([1, 512], fp32, tag="acc", name="p2")
                for dt in range(NDT):
                    if pre_sq is not None and pre_sq[dt] is not None:
                        sqt = pre_sq[dt]
                    else:
                        sqt = sq_p.tile([P, 512], fp32r, tag="sq", name="sqt")
                        nc.scalar.activation(
                            sqt, f(ytiles[dt][:, ssl]), AF.Square, bias=zero_col
                        )
                    mm(
                        p2,
                        invd_col,
                        sqt,
                        start=(dt == 0),
                        stop=(dt == NDT - 1),
                    )
                m_sb = rows_p.tile([1, 512], fp32, tag="m_sb")
                nc.scalar.copy(m_sb, p1)
                msq = rows_p.tile([1, 512], fp32, tag="msq")
                nc.scalar.activation(msq, m_sb, AF.Square, bias=zero_col[0:1])
                var = rows_p.tile([1, 512], fp32, tag="var")
                nc.vector.tensor_sub(var, p2, msq)
                # rstd = exp(-0.5 * ln(var + eps))
                nc.scalar.activation(var, var, AF.Ln, bias=eps_col[0:1])
                r_sb = rows_p.tile([1, 512], fp32r, tag="r_sb")
                nc.scalar.activation(
                    r_sb, var, AF.Exp, bias=zero_col[0:1], scale=-0.5
                )
                mrt = mr_ab[mr_idx]
                nc.vector.tensor_mul(mrt[0:1], m_sb, f(r_sb))
                bcr = bcs_p.tile([P, 512], fp32, tag="bcr", name="bcr")
                nc.gpsimd.partition_broadcast(bcr, f(r_sb), channels=P)
                return mrt, bcr

            def ln_finalize(ytiles, nh, g_t, gb_t, stats):
                # out = (y*g)*bc(r) + D,   D = (-g) (x) (m*r) + b  (k=2 MM)
                mrt, bcr = stats
                ssl = slice(nh * 512, (nh + 1) * 512)
                for dt in range(NDT):
                    Dt = pp_mm.tile([P, 512], fp32, tag="mm", name="Dt")
                    mm(Dt, gb_t[:, dt, :], mrt)
                    nc.vector.scalar_tensor_tensor(
                        ytiles[dt][:, ssl],
                        f(ytiles[dt][:, ssl]),
                        g_t[:, dt : dt + 1],
                        bcr,
                        OP.mult,
                        OP.mult,
                    )
                    nc.vector.tensor_add(
                        ytiles[dt][:, ssl], f(ytiles[dt][:, ssl]), Dt
                    )

            # state carried across the layer loop for LN2 pipelining
            carry = {}

            for l in range(L):
                # ---------------- weight loads (wv/wq/wk first) -------------
                wv_t = wbig_p.tile([P, NDT, H * DK], fp32r, tag="wv")
                nc.sync.dma_start(out=wv_t, in_=wv_d[l])
                wq_t = wbig_p.tile([P, NDT, NPAIR, P], fp32r, tag="wq")
                nc.sync.dma_start(out=wq_t, in_=wq_d[l])
                wk_t = wbig_p.tile([P, NDT, NPAIR, P], fp32r, tag="wk")
                nc.sync.dma_start(out=wk_t, in_=wk_d[l])
                g1_t = wts_p.tile([P, NDT], fp32, tag="g1")
                nc.sync.dma_start(out=g1_t, in_=g1_d[l])
                g2_t = wts_p.tile([P, NDT], fp32, tag="g2")
                nc.sync.dma_start(out=g2_t, in_=g2_d[l])
                gb1_t = wts_p.tile([2, NDT, P], fp32r, tag="gb1")
                nc.sync.dma_start(out=gb1_t, in_=gb1_d[l])
                gb2_t = wts_p.tile([2, NDT, P], fp32r, tag="gb2")
                nc.sync.dma_start(out=gb2_t, in_=gb2_d[l])
                b1_t = wts_p.tile([P, NFT], fp32, tag="b1")
                nc.sync.dma_start(out=b1_t, in_=b1_d[l])
                b2_t = wts_p.tile([P, NDT], fp32, tag="b2")
                nc.sync.dma_start(out=b2_t, in_=b2_d[l])
                wo_t = wbig_p.tile([P, NPAIR, NDT, P], bf16, tag="wo")
                nc.sync.dma_start(out=wo_t, in_=wo_d[l])
                w2_t = wbig_p.tile([P, NDT, NFT, P], bf16, tag="w2")
                nc.sync.dma_start(out=w2_t, in_=w2_d[l])

                # ---------------- prev-layer LN2 finalize, pipelined with V -
                if carry:
                    ln_finalize(xt, 0, carry["g2"], carry["gb2"], carry["s0"])

                # ---------------- V = x @ Wv  (col 64 = ones -> psum row 64 =
                # softmax denominator; rows 0..63 per head = ctx) ------------
                v_tiles = []
                for st in range(NST):
                    if st == 4 and carry:
                        ln_finalize(xt, 1, carry["g2"], carry["gb2"], carry["s1"])
                        carry.clear()
                    vt = v_p.tile([P, H, DK + 1], bf16, tag="v")
                    nc.sync.dma_start(out=vt[:, :, DK], in_=ones16_d[:, 0:H])
                    ps = pp_mm.tile([P, 512], fp32, tag="mm")
                    for dt in range(NDT):
                        mm(
                            ps,
                            xt[dt][:, st * P : (st + 1) * P],
                            wv_t[:, dt, :],
                            start=(dt == 0),
                            stop=(dt == NDT - 1),
                        )
                    nc.vector.tensor_copy(
                        vt[:, :, 0:DK], ps.rearrange("p (h k) -> p h k", h=H)
                    )
                    v_tiles.append(vt)
                if carry:  # layer 0 never reaches here with carry set
                    ln_finalize(xt, 1, carry["g2"], carry["gb2"], carry["s1"])
                    carry.clear()

                # ---------------- QKV (per head-pair, JIT) + attention ------
                qt = [None] * NPAIR
                kt = [None] * NPAIR
                ctx_pairs = []
                for pr in range(NPAIR):
                    cp = ctx_p.tile([P, S], bf16, tag="ctx", name="cp")
                    ctx_pairs.append(cp)

                def make_qk(pr, w_t, tag):
                    # PSUM->SBUF eviction rides ACT (GPSIMD can't read PSUM)
                    # so the attention-loop DVE burst doesn't delay the next
                    # pair's score matmuls; one more exp step moves to DVE in
                    # exchange (SCH_T has 3 entries).
                    dst = qk_p.tile([P, S], bf16, tag=tag)
                    for nh in range(NH):
                        ps = pp_mm.tile([P, 512], fp32, tag="mm")
                        for dt in range(NDT):
                            mm(
                                ps,
                                w_t[:, dt, pr, :],
                                xt[dt][:, nh * 512 : (nh + 1) * 512],
                                start=(dt == 0),
                                stop=(dt == NDT - 1),
                            )
                        nc.scalar.copy(dst[:, nh * 512 : (nh + 1) * 512], ps)
                    return dst

                qt[0] = make_qk(0, wq_t, "qt")
                kt[0] = make_qk(0, wk_t, "kt")
                for pr in range(NPAIR):
                    if pr + 1 < NPAIR:
                        # emit next pair's QKV before this pair's attention so
                        # its DVE evicts sit ahead of the attention DVE burst
                        qt[pr + 1] = make_qk(pr + 1, wq_t, "qt")
                        kt[pr + 1] = make_qk(pr + 1, wk_t, "kt")

                    hA, hB = 2 * pr, 2 * pr + 1
                    for nh in range(NH):
                        ssl = slice(nh * 512, (nh + 1) * 512)
                        psA = pp_acc.tile([65, 512], fp32, tag="acc")
                        psB = pp_acc.tile([65, 512], fp32, tag="acc")

                        es = [None] * NST

                        def emit_sc_exp(t):
                            # scores + exp for step t (emitted one step ahead
                            # of the ctx matmuls so the PE never waits on the
                            # exp engines)
                            tsl = slice(t * P, (t + 1) * P)
                            sc = pp_sc.tile([P, 1024], fp32, tag="sc")
                            mm(sc[:, 0:512], kt[pr][0:64, tsl], qt[pr][0:64, ssl])
                            mm(
                                sc[:, 512:1024],
                                kt[pr][64:128, tsl],
                                qt[pr][64:128, ssl],
                            )
                            if t in SCH_T:
                                # Schraudolph exp on DVE: bf16 bit pattern =
                                # int16(A*score + B); numerator and denominator
                                # share e, so softmax stays normalized.
                                ei = exp_p.tile([P, 1024], i16, tag="exp")
                                nc.vector.tensor_scalar(
                                    out=ei,
                                    in0=sc,
                                    scalar1=SCH_A,
                                    scalar2=SCH_B,
                                    op0=OP.mult,
                                    op1=OP.add,
                                )
                                es[t] = ei.bitcast(bf16)
                            else:
                                e = exp_p.tile([P, 1024], bf16, tag="exp")
                                nc.scalar.activation(
                                    e, sc, AF.Exp, bias=zero_col, scale=float(SCALE)
                                )
                                es[t] = e

                        emit_sc_exp(0)
                        for t in range(NST):
                            if t + 1 < NST:
                                emit_sc_exp(t + 1)
                            e = es[t]
                            mm(
                                psA,
                                v_tiles[t][:, hA, :],
                                e[:, 0:512],
                                start=(t == 0),
                                stop=(t == NST - 1),
                            )
                            mm(
                                psB,
                                v_tiles[t][:, hB, :],
                                e[:, 512:1024],
                                start=(t == 0),
                                stop=(t == NST - 1),
                            )
                        # normalize: denom at row 64, ctx rows 0..63.  The
                        # reciprocal lands at partition 0 of rX so the GPSIMD
                        # broadcast (which reads partition 0) can fan it out;
                        # ctx goes into the pair tile rows [64*h : 64*h+64].
                        for hoff, psX in ((0, psA), (64, psB)):
                            rX = rows_p.tile([1, 512], fp32, tag="rX")
                            nc.vector.reciprocal(rX, psX[64:65])
                            bcd = bcs_p.tile([64, 512], fp32, tag="bcd", name="bcd")
                            nc.gpsimd.partition_broadcast(bcd, rX, channels=64)
                            nc.vector.tensor_mul(
                                ctx_pairs[pr][hoff : hoff + 64, ssl],
                                psX[0:64],
                                bcd,
                            )

                # ---------------- Wo + residual -> y (pre-LN1), LN1 stats
                # pipelined between the halves ------------------------------
                y = []
                for mt in range(NDT):
                    yt = mha_p.tile([P, S], fp32r, tag="mha")
                    y.append(yt)
                stats1 = [None, None]
                for nh in range(NH):
                    ssl = slice(nh * 512, (nh + 1) * 512)
                    for mt in range(NDT):
                        if nh == 1 and mt == 1:
                            stats1[0] = ln_stats(y, 0, 0)
                        ps = pp_mm.tile([P, 512], fp32, tag="mm")
                        for pr in range(NPAIR):
                            mm(
                                ps,
                                wo_t[:, pr, mt, :],
                                ctx_pairs[pr][:, ssl],
                                start=(pr == 0),
                                stop=(pr == NPAIR - 1),
                            )
                        nc.vector.tensor_add(y[mt][:, ssl], ps, f(xt[mt][:, ssl]))
                ln_finalize(y, 0, g1_t, gb1_t, stats1[0])
                stats1[1] = ln_stats(y, 1, 1)

                # ---------------- FFN1 (nh-outer; W1 streamed per pass) -----
                f1s = [None] * NFT
                for nh in range(NH):
                    ssl = slice(nh * 512, (nh + 1) * 512)
                    for fc in range(NFT // FCH):
                        w1_t = w1c_p.tile([P, NDT, FCH, P], fp32r, tag="w1c")
                        nc.sync.dma_start(out=w1_t, in_=w1_d[l, fc])
                        for fi in range(FCH):
                            ft = fc * FCH + fi
                            if nh == 0 and ft == 3:
                                ln_finalize(y, 1, g1_t, gb1_t, stats1[1])
                            if nh == 0:
                                f1s[ft] = ff1_p.tile(
                                    [P, S], bf16, tag="ff1", name="f1"
                                )
                            ps = pp_sc.tile([P, 512], fp32, tag="sc", name="psf")
                            for dt in range(NDT):
                                mm(
                                    ps,
                                    w1_t[:, dt, fi, :],
                                    y[dt][:, ssl],
                                    start=(dt == 0),
                                    stop=(dt == NDT - 1),
                                )
                            nc.scalar.activation(
                                f1s[ft][:, ssl],
                                ps,
                                AF.Relu,
                                bias=b1_t[:, ft : ft + 1],
                            )

                # ---------------- FFN2 + residual -> z, LN2 stats pipelined -
                z = []
                for mt in range(NDT):
                    zt = xt_p.tile([P, S], fp32r, tag="xt")
                    z.append(zt)
                stats2 = [None, None]
                sq_nh1 = [None] * NDT
                for nh in range(NH):
                    ssl = slice(nh * 512, (nh + 1) * 512)
                    for mt in range(NDT):
                        if nh == 1 and mt == 1:
                            stats2[0] = ln_stats(z, 0, 0)
                        ff2 = pp_acc.tile([P, 512], fp32, tag="acc")
                        w = 512 // FF2_SPLIT
                        for hb in range(FF2_SPLIT):
                            for ft in range(NFT):
                                mm(
                                    ff2[:, hb * w : (hb + 1) * w],
                                    w2_t[:, mt, ft, :],
                                    f1s[ft][
                                        :, nh * 512 + hb * w : nh * 512 + (hb + 1) * w
                                    ],
                                    start=(ft == 0),
                                    stop=(ft == NFT - 1),
                                )
                        nc.vector.scalar_tensor_tensor(
                            z[mt][:, ssl],
                            ff2,
                            b2_t[:, mt : mt + 1],
                            f(y[mt][:, ssl]),
                            OP.add,
                            OP.add,
                        )
                        if nh == 1 and mt < NDT - 1:
                            # pre-square z (ACT queue only) so the LN2-nh1
                            # chain starts as soon as FFN2 retires
                            sqe = sq_p.tile([P, 512], fp32r, tag="sq", name="sqe")
                            nc.scalar.activation(
                                sqe, f(z[mt][:, ssl]), AF.Square, bias=zero_col
                            )
                            sq_nh1[mt] = sqe
                stats2[1] = ln_stats(z, 1, 1, pre_sq=sq_nh1)
                carry = {
                    "s0": stats2[0],
                    "s1": stats2[1],
                    "g2": g2_t,
                    "gb2": gb2_t,
                }
                xt = z

            # tail: finalize the last layer's LN2, streaming halves out
            ln_finalize(xt, 0, carry["g2"], carry["gb2"], carry["s0"])
            for dt in range(NDT):
                nc.sync.dma_start(
                    out=out_d[dt][:, 0:512], in_=f(xt[dt][:, 0:512])
                )
            ln_finalize(xt, 1, carry["g2"], carry["gb2"], carry["s1"])
            for dt in range(NDT):
                nc.sync.dma_start(
                    out=out_d[dt][:, 512:1024], in_=f(xt[dt][:, 512:1024])
                )

    return nc


def _prep_weights(Wq, Wk, Wv, Wo, ln1_g, ln1_b, W1, b1, W2, b2, ln2_g, ln2_b):
    import ml_dtypes

    f = np.float32
    bf = ml_dtypes.bfloat16

    def qk_r(W):  # [L,H,D,DK] -> [L, 128, NDT, NPAIR, 128] fp32r
        return _round_fp32r(
            W.reshape(L, NPAIR, 2, NDT, P, DK)
            .transpose(0, 4, 3, 1, 2, 5)
            .reshape(L, P, NDT, NPAIR, P)
            .astype(f)
        )

    wv_r = _round_fp32r(
        Wv.transpose(0, 2, 1, 3)  # [L, D, H, DK]
        .reshape(L, NDT, P, H * DK)
        .transpose(0, 2, 1, 3)
        .reshape(L, P, NDT, H * DK)
        .astype(f)
    )
    # Wo packed for k=128 head-pair steps: [l, c, pr, mt, fcol] where
    # c = (h_in_pair * 64 + dk) and the input feature is (2*pr+h_in_pair)*64+dk
    wo_r = (
        Wo.reshape(L, NPAIR, 2 * DK, NDT, P)
        .transpose(0, 2, 1, 3, 4)
        .astype(bf)
    )
    w1_r = _round_fp32r(
        W1.reshape(L, NDT, P, NFT // FCH, FCH, P)
        .transpose(0, 3, 2, 1, 4, 5)
        .astype(f)
    )
    # w2[l, p, mt, ft, fcol] = W2[l, ft*128+p, mt*128+fcol]
    w2_r = W2.reshape(L, NFT, P, NDT, P).transpose(0, 2, 3, 1, 4).astype(bf)

    def ln_r(v, n):  # [L, n*128] -> [L, 128, n]
        return np.ascontiguousarray(v.reshape(L, n, P).transpose(0, 2, 1).astype(f))

    def gb_r(g, b):  # [L, D] x2 -> [L, 2, NDT, P] rows (-g, b), fp32r
        gneg = (-np.asarray(g, dtype=f)).reshape(L, 1, NDT, P)
        bb = np.asarray(b, dtype=f).reshape(L, 1, NDT, P)
        return _round_fp32r(np.concatenate([gneg, bb], axis=1))

    return {
        "wq": qk_r(Wq),
        "wk": qk_r(Wk),
        "wv": wv_r,
        "wo": wo_r,
        "w1": w1_r,
        "w2": w2_r,
        "g1": ln_r(ln1_g, NDT),
        "g2": ln_r(ln2_g, NDT),
        "gb1": gb_r(ln1_g, ln1_b),
        "gb2": gb_r(ln2_g, ln2_b),
        "b1": ln_r(b1, NFT),
        "b2": ln_r(b2, NDT),
    }


def get_nc():
    if "nc" not in _CACHE:
        nc = _build_nc()
        if not nc.is_finalized():
            nc.finalize()
        _CACHE["nc"] = nc
    return _CACHE["nc"]


def make_in_maps(**inputs):
    import ml_dtypes

    inputs = {k: np.asarray(v, dtype=np.float32) for k, v in inputs.items()}
    x = inputs.pop("x")
    wmap = _prep_weights(**inputs)
    in_maps = []
    wmap["ones16"] = np.ones((P, 16), dtype=ml_dtypes.bfloat16)
    mro = np.zeros((2, 512), dtype=np.float32)
    mro[1, :] = 1.0
    wmap["mro"] = mro
    for b in range(B):
        xt = _round_fp32r(x[b].T.reshape(NDT, P, S))
        in_maps.append({"x": xt, **wmap})
    return in_maps


def kernel(**inputs) -> np.ndarray:
    from concourse.bass_utils import run_bass_kernel_spmd

    nc = get_nc()
    in_maps = make_in_maps(**inputs)
    res = run_bass_kernel_spmd(nc, in_maps, core_ids=list(range(B)))
    out = np.empty((B, S, D), dtype=np.float32)
    for b in range(B):
        out[b] = res.results[b]["out"].reshape(D, S).T
    return out


if __name__ == "__main__":
    rng = np.random.default_rng(0)
    ins = {
        "x": rng.standard_normal((B, S, D), dtype=np.float32),
        "Wq": rng.standard_normal((L, H, D, DK), dtype=np.float32) * 0.02,
        "Wk": rng.standard_normal((L, H, D, DK), dtype=np.float32) * 0.02,
        "Wv": rng.standard_normal((L, H, D, DK), dtype=np.float32) * 0.02,
        "Wo": rng.standard_normal((L, D, D), dtype=np.float32) * 0.02,
        "ln1_g": np.ones((L, D), np.float32),
        "ln1_b": np.zeros((L, D), np.float32),
        "W1": rng.standard_normal((L, D, DFF), dtype=np.float32) * 0.02,
        "b1": np.zeros((L, DFF), np.float32),
        "W2": rng.standard_normal((L, DFF, D), dtype=np.float32) * 0.02,
        "b2": np.zeros((L, D), np.float32),
        "ln2_g": np.ones((L, D), np.float32),
        "ln2_b": np.zeros((L, D), np.float32),
    }
    out = kernel(**ins)
    print(out.shape, out.dtype, np.abs(out).mean())
